# revision 23
# baseline (speedup 1.0000x reference)
"""Trainium2 Bass kernel for the DifferentiableRenderer problem.

Math: the reference splats N=8192 isotropic 2D gaussians onto a 128x128
pixel grid:  w[c,p] = op[c] * exp(-0.5*dist2(c,p)/var[c]),
             img = (w^T @ colors) / (sum_c w + eps).

Key algebraic restructuring: the pixel grid is separable, so
  exp(-0.5*((px-x)^2+(py-y)^2)/var) = gx[c,x] * gy[c,y]
with gx/gy 1D gaussian factors over the 128 grid coordinates.  The
accumulation then becomes, per channel d (3 colors + 1 denominator):
  num_d[y,x] = sum_c gy[c,y] * (gx*op*cl_d)[c,x]
i.e. a matmul contracting over gaussians, with only N*256 exps instead
of N*16384.

Sharding: gaussians are sharded 8 ways (1024/core); every core computes
partial (num, den) for the full image; host sums the 8 partials
(the "shard gaussians and all-reduce accumulators" strategy, with the
all-reduce realized as the host-side unshard of the partial outputs).

Device layout per core (1024 gaussians = 8 chunks of 128 partitions):
  gp  [128, 64]  per-gaussian params, chunk k at cols 8k..8k+7:
                 [px, py, -0.5/var, op*r, op*g, op*b, op, pad]
  xg  [128, 128] grid constant, xg[p, j] = j
  out [128, 512] partial accumulators: out[y, 128d+x], d=3 is den.

Pipeline per core:
  diff[c, k, {x,y}, 128] = grid - center   (2 wide DVE subs, broadcast APs)
  diff = diff*diff                          (2 wide DVE squares, in place)
  per chunk: gxy[c,256] = exp(iv*diff_k)    (1 ACT op, per-partition scale)
             B[c,4,128] = gx * opcl4        (1 DVE op, broadcast APs)
             acc += gyT @ B                 (1 PE matmul, float32r, PSUM acc)
"""

import numpy as np

H = W = 128
FX = FY = 150.0
CX = CY = 64.0
N = 8192
NCORES = 8
NPC = N // NCORES          # gaussians per core
NCHUNK = NPC // 128        # 128-gaussian chunks per core
REF_CHUNK_EPS = (N // 2048) * 1e-8  # reference adds EPS to den once per chunk step
EPS = 1e-8

_prog_cache = {}


def _make_split_wait_tile_context(tile_mod):
    """TileContext whose commit stage splits multi-wait instructions.

    The walrus build used on the axon path allows at most ONE sync wait
    per instruction (any engine/format).  Tile's add_semaphores pass
    freely emits 2-3 waits per instruction, so at commit time we peel
    all but the last wait onto standalone EventSemaphore instructions
    appended just before the real one on the same engine queue —
    semantically identical (waits on one queue execute in order)."""
    import bass_rust
    from concourse import mybir

    class SplitWaitTileContext(tile_mod.TileContext):
        def _add_instruction(self, inst):
            si = inst.sync_info
            if si is not None and si.on_wait and len(si.on_wait) > 1:
                waits = list(si.on_wait)
                for j, w in enumerate(waits[:-1]):
                    carrier = mybir.InstEventSemaphore(
                        name=f"{inst.name}-sw{j}",
                        ins=[],
                        outs=[],
                        engine=inst.engine,
                        sync_info=bass_rust.SyncInfo(on_wait=[w], on_update=[]),
                    )
                    super()._add_instruction(carrier)
                si.on_wait = [waits[-1]]
            super()._add_instruction(inst)

    return SplitWaitTileContext


def _strip_ctrl_drain_waits(nc):
    # The tail drain aggregates one wait per open semaphore, but CTRL
    # instructions accept only one; the waits are redundant at kernel
    # end (the all-engine barrier that follows drains every engine and
    # its DMA rings before semaphores are cleared).  Keep one.
    for blk in nc.m.functions[0].blocks:
        for inst in blk.instructions:
            si = inst.sync_info
            if (
                si is not None
                and si.on_wait
                and len(si.on_wait) > 1
                and type(inst).__name__ == "InstDrain"
            ):
                waits = list(si.on_wait)
                keep = [w for w in waits if not w.ant_name.startswith("DMA")][:1]
                if not keep:
                    keep = waits[:1]
                si.on_wait = keep


def _build_program():
    import concourse.bass as bass
    import concourse.tile as tile
    from concourse import mybir

    f32 = mybir.dt.float32
    f16 = mybir.dt.float16
    Exp = mybir.ActivationFunctionType.Exp
    sub_op = mybir.AluOpType.subtract
    mul_op = mybir.AluOpType.mult

    nc = bass.Bass(debug=False)
    gp = nc.dram_tensor("gp", [128, 4 * NCHUNK], f32, kind="ExternalInput")
    gph = nc.dram_tensor("gph", [128, 3 * NCHUNK], f16, kind="ExternalInput")
    out = nc.dram_tensor("out", [128, 512], f32, kind="ExternalOutput")

    TC = _make_split_wait_tile_context(tile)
    with TC(nc) as tc:
        with (
            tc.tile_pool(name="const", bufs=1) as cpool,
            tc.tile_pool(name="work", bufs=4) as wpool,
            tc.tile_pool(name="psum", bufs=1, space="PSUM") as ppool,
        ):
            # Dummy exp on a const input: pulls the ACT exp-table load off
            # the critical path (it otherwise serializes before chunk 0).
            warm = cpool.tile([128, 1], f32)
            nc.scalar.activation(
                warm[:], nc.const_aps.scalar_like(0.0, warm[:]), Exp
            )

            gp_t = cpool.tile([128, 4 * NCHUNK], f32)
            nc.gpsimd.dma_start(gp_t[:], gp[:])
            # grid constant generated on device: xg[p, j] = j  (before the
            # gph DMA on the gpsimd queue: xg gates the first sub, gph
            # is not needed until the first B-form)
            xg_t = cpool.tile([128, 128], f32)
            nc.gpsimd.iota(
                xg_t[:],
                pattern=[[1, 128]],
                channel_multiplier=0,
                allow_small_or_imprecise_dtypes=True,
            )
            gph_t = cpool.tile([128, 3 * NCHUNK], f16)
            nc.sync.dma_start(gph_t[:], gph[:])

            gp_v = gp_t[:].rearrange("p (k f) -> p k f", f=4)  # [128, 8, 4]

            # diff[c, k, {x,y}, 128]: subs + squares on DVE (GpSimd
            # elementwise halves DVE throughput via port sharing), in
            # 2-chunk quarters so chunk 0's exp starts early.
            diff = cpool.tile([128, NCHUNK, 2, 128], f32)
            QC = 2
            for h in range(NCHUNK // QC):
                ks = slice(h * QC, (h + 1) * QC)
                nc.vector.tensor_tensor(
                    diff[:, ks],
                    xg_t[:, None, None, :].broadcast_to([128, QC, 2, 128]),
                    gp_v[:, ks, 0:2, None].broadcast_to([128, QC, 2, 128]),
                    sub_op,
                )
                nc.vector.tensor_tensor(diff[:, ks], diff[:, ks], diff[:, ks], mul_op)

            gph_v = gph_t[:].rearrange("p (k f) -> p k f", f=3)  # [128, 8, 3]
            acc = ppool.tile([128, 384], f32)
            accd = ppool.tile([128, 128], f32)
            for k2 in range(NCHUNK // 2):
                # fp16 gaussian factors; opacity folded into gy via the
                # log-opacity activation bias, so B only needs the 3 color
                # channels and the den channel is gx itself (2nd matmul).
                gxy = wpool.tile([128, 2, 256], f16, tag="gxy")
                for j in range(2):
                    k = 2 * k2 + j
                    nc.scalar.activation(
                        gxy[:, j],
                        diff[:, k].rearrange("p a b -> p (a b)"),
                        Exp,
                        bias=gp_t[:, 4 * k + 3 : 4 * k + 4],
                        scale=gp_t[:, 4 * k + 2 : 4 * k + 3],
                    )
                B = wpool.tile([128, 2, 3, 128], f16, tag="B")
                nc.vector.tensor_tensor(
                    B[:],
                    gxy[:, :, None, 0:128].broadcast_to([128, 2, 3, 128]),
                    gph_v[:, 2 * k2 : 2 * k2 + 2, :, None].broadcast_to(
                        [128, 2, 3, 128]
                    ),
                    mul_op,
                )
                for j in range(2):
                    k = 2 * k2 + j
                    gy = gxy[:, j, 128:256]
                    nc.tensor.matmul(
                        accd[:],
                        gy,
                        gxy[:, j, 0:128],
                        start=(k == 0),
                        stop=(k == NCHUNK - 1),
                    )
                    nc.tensor.matmul(
                        acc[:],
                        gy,
                        B[:, j].rearrange("p a b -> p (a b)"),
                        start=(k == 0),
                        stop=(k == NCHUNK - 1),
                    )

            # fp16 partials out; den finishes ~2us before num (its last
            # accumulating matmul is earlier in the PE queue), so copy +
            # DMA it while the num matmuls still run.
            out_sb = cpool.tile([128, 512], f32)
            nc.vector.tensor_copy(out_sb[:, 384:512], accd[:])
            nc.sync.dma_start(out[:, 384:512], out_sb[:, 384:512])
            nc.vector.tensor_copy(out_sb[:, 0:384], acc[:])
            nc.sync.dma_start(out[:, 0:384], out_sb[:, 0:384])

    _strip_ctrl_drain_waits(nc)
    return nc


def _get_program():
    if "nc" not in _prog_cache:
        _prog_cache["nc"] = _build_program()
    return _prog_cache["nc"]


def _host_preprocess(positions, colors, opacities, scales, qvec, tvec):
    # Mirror the reference's fp32 projection math.
    q = qvec.astype(np.float32)
    q = q / np.sqrt(np.sum(q * q, dtype=np.float32)).astype(np.float32)
    w, x, y, z = q[0], q[1], q[2], q[3]
    R = np.array(
        [
            [1 - 2 * (y * y + z * z), 2 * (x * y - z * w), 2 * (x * z + y * w)],
            [2 * (x * y + z * w), 1 - 2 * (x * x + z * z), 2 * (y * z - x * w)],
            [2 * (x * z - y * w), 2 * (y * z + x * w), 1 - 2 * (x * x + y * y)],
        ],
        dtype=np.float32,
    )
    p_cam = positions.astype(np.float32) @ R.T + tvec.astype(np.float32)[None, :]
    px = p_cam[:, 0] / p_cam[:, 2] * np.float32(FX) + np.float32(CX)
    py = p_cam[:, 1] / p_cam[:, 2] * np.float32(FY) + np.float32(CY)

    var = scales[:, 0].astype(np.float32) ** 2
    iv = np.float32(-0.5) / var
    op = opacities[:, 0].astype(np.float32)

    # gp[c] = [px, py, iv, log(op), ...]; gph[c] = fp16 colors [r, g, b]
    gp = np.zeros((N, 4), dtype=np.float32)
    gp[:, 0] = px
    gp[:, 1] = py
    gp[:, 2] = iv
    # exp bias applies to both the gx and gy halves of the per-chunk
    # activation, so each factor carries sqrt(op): gx*gy = op*...
    gp[:, 3] = np.float32(0.5) * np.log(np.maximum(op, np.float32(1e-38)))
    gph = colors.astype(np.float16)
    # core/chunk/partition layout: [NCORES, NCHUNK, 128, f] -> [cores][128, NCHUNK*f]
    gp = gp.reshape(NCORES, NCHUNK, 128, 4).transpose(0, 2, 1, 3)
    gph = gph.reshape(NCORES, NCHUNK, 128, 3).transpose(0, 2, 1, 3)
    return (
        np.ascontiguousarray(gp.reshape(NCORES, 128, NCHUNK * 4)),
        np.ascontiguousarray(gph.reshape(NCORES, 128, NCHUNK * 3)),
    )


def _host_postprocess(partials):
    # partials: [NCORES, 128(y), 512] -> full image
    tot = partials.sum(axis=0, dtype=np.float32)  # [y, 512]
    num = tot[:, :384].reshape(128, 3, 128).transpose(0, 2, 1)  # [y, x, 3]
    den = tot[:, 384:512] + np.float32(REF_CHUNK_EPS)  # [y, x]
    img = num / np.maximum(den, np.float32(EPS))[:, :, None]
    img_flat = img.reshape(H * W, 3)
    step_px = 64 * 64
    n_tiles = (H * W) // step_px
    tiles = img_flat.reshape(n_tiles, step_px, 3).transpose(0, 2, 1)
    return np.ascontiguousarray(tiles.reshape(n_tiles, 3, 64, 64), dtype=np.float32)


def kernel(positions, colors, opacities, scales, qvec, tvec):
    from concourse.bass_utils import run_bass_kernel_spmd

    positions = np.asarray(positions)
    colors = np.asarray(colors)
    opacities = np.asarray(opacities)
    scales = np.asarray(scales)
    qvec = np.asarray(qvec)
    tvec = np.asarray(tvec)

    nc = _get_program()
    gp_per_core, gph_per_core = _host_preprocess(
        positions, colors, opacities, scales, qvec, tvec
    )
    in_maps = [
        {"gp": gp_per_core[i], "gph": gph_per_core[i]} for i in range(NCORES)
    ]
    res = run_bass_kernel_spmd(nc, in_maps, list(range(NCORES)))
    partials = np.stack(
        [res.results[i]["out"].astype(np.float32) for i in range(NCORES)]
    )
    return _host_postprocess(partials)


# revision 24
# speedup vs baseline: 1.0083x; 1.0083x over previous
"""Trainium2 Bass kernel for the DifferentiableRenderer problem.

Math: the reference splats N=8192 isotropic 2D gaussians onto a 128x128
pixel grid:  w[c,p] = op[c] * exp(-0.5*dist2(c,p)/var[c]),
             img = (w^T @ colors) / (sum_c w + eps).

Key algebraic restructuring: the pixel grid is separable, so
  exp(-0.5*((px-x)^2+(py-y)^2)/var) = gx[c,x] * gy[c,y]
with gx/gy 1D gaussian factors over the 128 grid coordinates.  The
accumulation then becomes, per channel d (3 colors + 1 denominator):
  num_d[y,x] = sum_c gy[c,y] * (gx*op*cl_d)[c,x]
i.e. a matmul contracting over gaussians, with only N*256 exps instead
of N*16384.

Sharding: gaussians are sharded 8 ways (1024/core); every core computes
partial (num, den) for the full image; host sums the 8 partials
(the "shard gaussians and all-reduce accumulators" strategy, with the
all-reduce realized as the host-side unshard of the partial outputs).

Device layout per core (1024 gaussians = 8 chunks of 128 partitions):
  gp  [128, 64]  per-gaussian params, chunk k at cols 8k..8k+7:
                 [px, py, -0.5/var, op*r, op*g, op*b, op, pad]
  xg  [128, 128] grid constant, xg[p, j] = j
  out [128, 512] partial accumulators: out[y, 128d+x], d=3 is den.

Pipeline per core:
  diff[c, k, {x,y}, 128] = grid - center   (2 wide DVE subs, broadcast APs)
  diff = diff*diff                          (2 wide DVE squares, in place)
  per chunk: gxy[c,256] = exp(iv*diff_k)    (1 ACT op, per-partition scale)
             B[c,4,128] = gx * opcl4        (1 DVE op, broadcast APs)
             acc += gyT @ B                 (1 PE matmul, float32r, PSUM acc)
"""

import numpy as np

H = W = 128
FX = FY = 150.0
CX = CY = 64.0
N = 8192
NCORES = 8
NPC = N // NCORES          # gaussians per core
NCHUNK = NPC // 128        # 128-gaussian chunks per core
REF_CHUNK_EPS = (N // 2048) * 1e-8  # reference adds EPS to den once per chunk step
EPS = 1e-8

_prog_cache = {}


def _make_split_wait_tile_context(tile_mod):
    """TileContext whose commit stage splits multi-wait instructions.

    The walrus build used on the axon path allows at most ONE sync wait
    per instruction (any engine/format).  Tile's add_semaphores pass
    freely emits 2-3 waits per instruction, so at commit time we peel
    all but the last wait onto standalone EventSemaphore instructions
    appended just before the real one on the same engine queue —
    semantically identical (waits on one queue execute in order)."""
    import bass_rust
    from concourse import mybir

    class SplitWaitTileContext(tile_mod.TileContext):
        def _add_instruction(self, inst):
            si = inst.sync_info
            if si is not None and si.on_wait and len(si.on_wait) > 1:
                waits = list(si.on_wait)
                for j, w in enumerate(waits[:-1]):
                    carrier = mybir.InstEventSemaphore(
                        name=f"{inst.name}-sw{j}",
                        ins=[],
                        outs=[],
                        engine=inst.engine,
                        sync_info=bass_rust.SyncInfo(on_wait=[w], on_update=[]),
                    )
                    super()._add_instruction(carrier)
                si.on_wait = [waits[-1]]
            super()._add_instruction(inst)

    return SplitWaitTileContext


def _strip_ctrl_drain_waits(nc):
    # The tail drain aggregates one wait per open semaphore, but CTRL
    # instructions accept only one; the waits are redundant at kernel
    # end (the all-engine barrier that follows drains every engine and
    # its DMA rings before semaphores are cleared).  Keep one.
    for blk in nc.m.functions[0].blocks:
        for inst in blk.instructions:
            si = inst.sync_info
            if (
                si is not None
                and si.on_wait
                and len(si.on_wait) > 1
                and type(inst).__name__ == "InstDrain"
            ):
                waits = list(si.on_wait)
                keep = [w for w in waits if not w.ant_name.startswith("DMA")][:1]
                if not keep:
                    keep = waits[:1]
                si.on_wait = keep


def _build_program():
    import concourse.bass as bass
    import concourse.tile as tile
    from concourse import mybir

    f32 = mybir.dt.float32
    f16 = mybir.dt.float16
    Exp = mybir.ActivationFunctionType.Exp
    sub_op = mybir.AluOpType.subtract
    mul_op = mybir.AluOpType.mult

    nc = bass.Bass(debug=False)
    gp = nc.dram_tensor("gp", [128, 4 * NCHUNK], f32, kind="ExternalInput")
    gph = nc.dram_tensor("gph", [128, 3 * NCHUNK], f16, kind="ExternalInput")
    out = nc.dram_tensor("out", [128, 512], f32, kind="ExternalOutput")

    TC = _make_split_wait_tile_context(tile)
    with TC(nc) as tc:
        with (
            tc.tile_pool(name="const", bufs=1) as cpool,
            tc.tile_pool(name="work", bufs=4) as wpool,
            tc.tile_pool(name="psum", bufs=1, space="PSUM") as ppool,
        ):
            # Dummy exp on a const input: pulls the ACT exp-table load off
            # the critical path (it otherwise serializes before chunk 0).
            warm = cpool.tile([128, 1], f32)
            nc.scalar.activation(
                warm[:], nc.const_aps.scalar_like(0.0, warm[:]), Exp
            )

            gp_t = cpool.tile([128, 4 * NCHUNK], f32)
            nc.sync.dma_start(gp_t[:], gp[:])
            # grid constant generated on device: xg[p, j] = j  (before the
            # gph DMA on the gpsimd queue: xg gates the first sub, gph
            # is not needed until the first B-form)
            xg_t = cpool.tile([128, 128], f32)
            nc.gpsimd.iota(
                xg_t[:],
                pattern=[[1, 128]],
                channel_multiplier=0,
                allow_small_or_imprecise_dtypes=True,
            )
            gph_t = cpool.tile([128, 3 * NCHUNK], f16)
            nc.gpsimd.dma_start(gph_t[:], gph[:])

            gp_v = gp_t[:].rearrange("p (k f) -> p k f", f=4)  # [128, 8, 4]

            # diff[c, k, {x,y}, 128]: subs + squares on DVE (GpSimd
            # elementwise halves DVE throughput via port sharing), in
            # 2-chunk quarters so chunk 0's exp starts early.
            diff = cpool.tile([128, NCHUNK, 2, 128], f32)
            QC = 2
            for h in range(NCHUNK // QC):
                ks = slice(h * QC, (h + 1) * QC)
                nc.vector.tensor_tensor(
                    diff[:, ks],
                    xg_t[:, None, None, :].broadcast_to([128, QC, 2, 128]),
                    gp_v[:, ks, 0:2, None].broadcast_to([128, QC, 2, 128]),
                    sub_op,
                )
                nc.vector.tensor_tensor(diff[:, ks], diff[:, ks], diff[:, ks], mul_op)

            gph_v = gph_t[:].rearrange("p (k f) -> p k f", f=3)  # [128, 8, 3]
            acc = ppool.tile([128, 384], f32)
            accd = ppool.tile([128, 128], f32)
            for k2 in range(NCHUNK // 2):
                # fp16 gaussian factors; opacity folded into gy via the
                # log-opacity activation bias, so B only needs the 3 color
                # channels and the den channel is gx itself (2nd matmul).
                gxy = wpool.tile([128, 2, 256], f16, tag="gxy")
                for j in range(2):
                    k = 2 * k2 + j
                    nc.scalar.activation(
                        gxy[:, j],
                        diff[:, k].rearrange("p a b -> p (a b)"),
                        Exp,
                        bias=gp_t[:, 4 * k + 3 : 4 * k + 4],
                        scale=gp_t[:, 4 * k + 2 : 4 * k + 3],
                    )
                B = wpool.tile([128, 2, 3, 128], f16, tag="B")
                nc.vector.tensor_tensor(
                    B[:],
                    gxy[:, :, None, 0:128].broadcast_to([128, 2, 3, 128]),
                    gph_v[:, 2 * k2 : 2 * k2 + 2, :, None].broadcast_to(
                        [128, 2, 3, 128]
                    ),
                    mul_op,
                )
                for j in range(2):
                    k = 2 * k2 + j
                    gy = gxy[:, j, 128:256]
                    nc.tensor.matmul(
                        accd[:],
                        gy,
                        gxy[:, j, 0:128],
                        start=(k == 0),
                        stop=(k == NCHUNK - 1),
                    )
                    nc.tensor.matmul(
                        acc[:],
                        gy,
                        B[:, j].rearrange("p a b -> p (a b)"),
                        start=(k == 0),
                        stop=(k == NCHUNK - 1),
                    )

            # fp16 partials out; den finishes ~2us before num (its last
            # accumulating matmul is earlier in the PE queue), so copy +
            # DMA it while the num matmuls still run.
            out_sb = cpool.tile([128, 512], f32)
            nc.vector.tensor_copy(out_sb[:, 384:512], accd[:])
            nc.sync.dma_start(out[:, 384:512], out_sb[:, 384:512])
            nc.vector.tensor_copy(out_sb[:, 0:384], acc[:])
            nc.sync.dma_start(out[:, 0:384], out_sb[:, 0:384])

    _strip_ctrl_drain_waits(nc)
    return nc


def _get_program():
    if "nc" not in _prog_cache:
        _prog_cache["nc"] = _build_program()
    return _prog_cache["nc"]


def _host_preprocess(positions, colors, opacities, scales, qvec, tvec):
    # Mirror the reference's fp32 projection math.
    q = qvec.astype(np.float32)
    q = q / np.sqrt(np.sum(q * q, dtype=np.float32)).astype(np.float32)
    w, x, y, z = q[0], q[1], q[2], q[3]
    R = np.array(
        [
            [1 - 2 * (y * y + z * z), 2 * (x * y - z * w), 2 * (x * z + y * w)],
            [2 * (x * y + z * w), 1 - 2 * (x * x + z * z), 2 * (y * z - x * w)],
            [2 * (x * z - y * w), 2 * (y * z + x * w), 1 - 2 * (x * x + y * y)],
        ],
        dtype=np.float32,
    )
    p_cam = positions.astype(np.float32) @ R.T + tvec.astype(np.float32)[None, :]
    px = p_cam[:, 0] / p_cam[:, 2] * np.float32(FX) + np.float32(CX)
    py = p_cam[:, 1] / p_cam[:, 2] * np.float32(FY) + np.float32(CY)

    var = scales[:, 0].astype(np.float32) ** 2
    iv = np.float32(-0.5) / var
    op = opacities[:, 0].astype(np.float32)

    # gp[c] = [px, py, iv, log(op), ...]; gph[c] = fp16 colors [r, g, b]
    gp = np.zeros((N, 4), dtype=np.float32)
    gp[:, 0] = px
    gp[:, 1] = py
    gp[:, 2] = iv
    # exp bias applies to both the gx and gy halves of the per-chunk
    # activation, so each factor carries sqrt(op): gx*gy = op*...
    gp[:, 3] = np.float32(0.5) * np.log(np.maximum(op, np.float32(1e-38)))
    gph = colors.astype(np.float16)
    # core/chunk/partition layout: [NCORES, NCHUNK, 128, f] -> [cores][128, NCHUNK*f]
    gp = gp.reshape(NCORES, NCHUNK, 128, 4).transpose(0, 2, 1, 3)
    gph = gph.reshape(NCORES, NCHUNK, 128, 3).transpose(0, 2, 1, 3)
    return (
        np.ascontiguousarray(gp.reshape(NCORES, 128, NCHUNK * 4)),
        np.ascontiguousarray(gph.reshape(NCORES, 128, NCHUNK * 3)),
    )


def _host_postprocess(partials):
    # partials: [NCORES, 128(y), 512] -> full image
    tot = partials.sum(axis=0, dtype=np.float32)  # [y, 512]
    num = tot[:, :384].reshape(128, 3, 128).transpose(0, 2, 1)  # [y, x, 3]
    den = tot[:, 384:512] + np.float32(REF_CHUNK_EPS)  # [y, x]
    img = num / np.maximum(den, np.float32(EPS))[:, :, None]
    img_flat = img.reshape(H * W, 3)
    step_px = 64 * 64
    n_tiles = (H * W) // step_px
    tiles = img_flat.reshape(n_tiles, step_px, 3).transpose(0, 2, 1)
    return np.ascontiguousarray(tiles.reshape(n_tiles, 3, 64, 64), dtype=np.float32)


def kernel(positions, colors, opacities, scales, qvec, tvec):
    from concourse.bass_utils import run_bass_kernel_spmd

    positions = np.asarray(positions)
    colors = np.asarray(colors)
    opacities = np.asarray(opacities)
    scales = np.asarray(scales)
    qvec = np.asarray(qvec)
    tvec = np.asarray(tvec)

    nc = _get_program()
    gp_per_core, gph_per_core = _host_preprocess(
        positions, colors, opacities, scales, qvec, tvec
    )
    in_maps = [
        {"gp": gp_per_core[i], "gph": gph_per_core[i]} for i in range(NCORES)
    ]
    res = run_bass_kernel_spmd(nc, in_maps, list(range(NCORES)))
    partials = np.stack(
        [res.results[i]["out"].astype(np.float32) for i in range(NCORES)]
    )
    return _host_postprocess(partials)


# revision 25
# speedup vs baseline: 1.0208x; 1.0124x over previous
"""Trainium2 Bass kernel for the DifferentiableRenderer problem.

Math: the reference splats N=8192 isotropic 2D gaussians onto a 128x128
pixel grid:  w[c,p] = op[c] * exp(-0.5*dist2(c,p)/var[c]),
             img = (w^T @ colors) / (sum_c w + eps).

Key algebraic restructuring: the pixel grid is separable, so
  exp(-0.5*((px-x)^2+(py-y)^2)/var) = gx[c,x] * gy[c,y]
with gx/gy 1D gaussian factors over the 128 grid coordinates.  The
accumulation then becomes, per channel d (3 colors + 1 denominator):
  num_d[y,x] = sum_c gy[c,y] * (gx*op*cl_d)[c,x]
i.e. a matmul contracting over gaussians, with only N*256 exps instead
of N*16384.

Sharding: gaussians are sharded 8 ways (1024/core); every core computes
partial (num, den) for the full image; host sums the 8 partials
(the "shard gaussians and all-reduce accumulators" strategy, with the
all-reduce realized as the host-side unshard of the partial outputs).

Device layout per core (1024 gaussians = 8 chunks of 128 partitions):
  gp  [128, 32]  fp32 params, chunk k at cols 4k..4k+3:
                 [px, py, -0.5/var, 0.5*log(op)]
  gph [128, 24]  fp16 colors, chunk k at cols 3k..3k+2: [r, g, b]
  out [128, 512] fp32 partial accumulators out[y, 128d+x], d=3 is den.
  (the pixel-grid constant xg[p, j] = j is generated on device via iota)

Pipeline per core:
  diff[c, k, {x,y}, 128] = (grid - center)^2   (DVE TTs, broadcast APs,
      2-chunk quarters so chunk 0's exp starts early)
  per chunk: gxy[c, 2, 256] = exp(iv*diff_k + log(op)/2)  fp16
      (1 ACT op per chunk; the bias hits both the gx and gy halves, so
      each carries sqrt(op))
  per chunk pair: B[c, 2, 3, 128] = gx * rgb   (1 DVE TT, broadcast APs)
  per chunk: accd[y, 128]  += gy^T @ gx        (PE fp16, den PSUM bank)
             acc[y, 384]   += gy^T @ B_k       (PE fp16, num PSUM bank)
  copy PSUM->SBUF, den half DMA'd out early, num half after last matmul.
"""

import numpy as np

H = W = 128
FX = FY = 150.0
CX = CY = 64.0
N = 8192
NCORES = 8
NPC = N // NCORES          # gaussians per core
NCHUNK = NPC // 128        # 128-gaussian chunks per core
REF_CHUNK_EPS = (N // 2048) * 1e-8  # reference adds EPS to den once per chunk step
EPS = 1e-8

_prog_cache = {}


def _make_split_wait_tile_context(tile_mod):
    """TileContext whose commit stage splits multi-wait instructions.

    The walrus build used on the axon path allows at most ONE sync wait
    per instruction (any engine/format).  Tile's add_semaphores pass
    freely emits 2-3 waits per instruction, so at commit time we peel
    all but the last wait onto standalone EventSemaphore instructions
    appended just before the real one on the same engine queue —
    semantically identical (waits on one queue execute in order)."""
    import bass_rust
    from concourse import mybir

    class SplitWaitTileContext(tile_mod.TileContext):
        def _add_instruction(self, inst):
            si = inst.sync_info
            if si is not None and si.on_wait and len(si.on_wait) > 1:
                waits = list(si.on_wait)
                for j, w in enumerate(waits[:-1]):
                    carrier = mybir.InstEventSemaphore(
                        name=f"{inst.name}-sw{j}",
                        ins=[],
                        outs=[],
                        engine=inst.engine,
                        sync_info=bass_rust.SyncInfo(on_wait=[w], on_update=[]),
                    )
                    super()._add_instruction(carrier)
                si.on_wait = [waits[-1]]
            super()._add_instruction(inst)

    return SplitWaitTileContext


def _strip_ctrl_drain_waits(nc):
    # The tail drain aggregates one wait per open semaphore, but CTRL
    # instructions accept only one; the waits are redundant at kernel
    # end (the all-engine barrier that follows drains every engine and
    # its DMA rings before semaphores are cleared).  Keep one.
    for blk in nc.m.functions[0].blocks:
        for inst in blk.instructions:
            si = inst.sync_info
            if (
                si is not None
                and si.on_wait
                and len(si.on_wait) > 1
                and type(inst).__name__ == "InstDrain"
            ):
                waits = list(si.on_wait)
                keep = [w for w in waits if not w.ant_name.startswith("DMA")][:1]
                if not keep:
                    keep = waits[:1]
                si.on_wait = keep


def _build_program():
    import concourse.bass as bass
    import concourse.tile as tile
    from concourse import mybir

    f32 = mybir.dt.float32
    f16 = mybir.dt.float16
    Exp = mybir.ActivationFunctionType.Exp
    sub_op = mybir.AluOpType.subtract
    mul_op = mybir.AluOpType.mult

    nc = bass.Bass(debug=False)
    gp = nc.dram_tensor("gp", [128, 4 * NCHUNK], f32, kind="ExternalInput")
    gph = nc.dram_tensor("gph", [128, 3 * NCHUNK], f16, kind="ExternalInput")
    out = nc.dram_tensor("out", [128, 512], f32, kind="ExternalOutput")

    TC = _make_split_wait_tile_context(tile)
    with TC(nc) as tc:
        with (
            tc.tile_pool(name="const", bufs=1) as cpool,
            tc.tile_pool(name="work", bufs=4) as wpool,
            tc.tile_pool(name="psum", bufs=1, space="PSUM") as ppool,
        ):
            # Dummy exp on a const input: pulls the ACT exp-table load off
            # the critical path (it otherwise serializes before chunk 0).
            warm = cpool.tile([128, 1], f32)
            nc.scalar.activation(
                warm[:], nc.const_aps.scalar_like(0.0, warm[:]), Exp
            )

            gp_t = cpool.tile([128, 4 * NCHUNK], f32)
            nc.sync.dma_start(gp_t[:], gp[:])
            # grid constant generated on device: xg[p, j] = j  (before the
            # gph DMA on the gpsimd queue: xg gates the first sub, gph
            # is not needed until the first B-form)
            xg_t = cpool.tile([128, 128], f32)
            nc.gpsimd.iota(
                xg_t[:],
                pattern=[[1, 128]],
                channel_multiplier=0,
                allow_small_or_imprecise_dtypes=True,
            )
            gph_t = cpool.tile([128, 3 * NCHUNK], f16)
            nc.gpsimd.dma_start(gph_t[:], gph[:])

            gp_v = gp_t[:].rearrange("p (k f) -> p k f", f=4)  # [128, 8, 4]

            # diff[c, k, {x,y}, 128]: subs + squares on DVE (GpSimd
            # elementwise halves DVE throughput via port sharing), in
            # 2-chunk quarters so chunk 0's exp starts early.
            diff = cpool.tile([128, NCHUNK, 2, 128], f32)
            QC = 2
            for h in range(NCHUNK // QC):
                ks = slice(h * QC, (h + 1) * QC)
                nc.vector.tensor_tensor(
                    diff[:, ks],
                    xg_t[:, None, None, :].broadcast_to([128, QC, 2, 128]),
                    gp_v[:, ks, 0:2, None].broadcast_to([128, QC, 2, 128]),
                    sub_op,
                )
                nc.vector.tensor_tensor(diff[:, ks], diff[:, ks], diff[:, ks], mul_op)

            gph_v = gph_t[:].rearrange("p (k f) -> p k f", f=3)  # [128, 8, 3]
            acc = ppool.tile([128, 384], f32)
            accd = ppool.tile([128, 128], f32)
            for k2 in range(NCHUNK // 2):
                # fp16 gaussian factors; opacity folded into gy via the
                # log-opacity activation bias, so B only needs the 3 color
                # channels and the den channel is gx itself (2nd matmul).
                gxy = wpool.tile([128, 2, 256], f16, tag="gxy")
                for j in range(2):
                    k = 2 * k2 + j
                    nc.scalar.activation(
                        gxy[:, j],
                        diff[:, k].rearrange("p a b -> p (a b)"),
                        Exp,
                        bias=gp_t[:, 4 * k + 3 : 4 * k + 4],
                        scale=gp_t[:, 4 * k + 2 : 4 * k + 3],
                    )
                B = wpool.tile([128, 2, 3, 128], f16, tag="B")
                nc.vector.tensor_tensor(
                    B[:],
                    gxy[:, :, None, 0:128].broadcast_to([128, 2, 3, 128]),
                    gph_v[:, 2 * k2 : 2 * k2 + 2, :, None].broadcast_to(
                        [128, 2, 3, 128]
                    ),
                    mul_op,
                )
                for j in range(2):
                    k = 2 * k2 + j
                    gy = gxy[:, j, 128:256]
                    nc.tensor.matmul(
                        accd[:],
                        gy,
                        gxy[:, j, 0:128],
                        start=(k == 0),
                        stop=(k == NCHUNK - 1),
                    )
                    nc.tensor.matmul(
                        acc[:],
                        gy,
                        B[:, j].rearrange("p a b -> p (a b)"),
                        start=(k == 0),
                        stop=(k == NCHUNK - 1),
                    )

            # fp16 partials out; den finishes ~2us before num (its last
            # accumulating matmul is earlier in the PE queue), so copy +
            # DMA it while the num matmuls still run.
            out_sb = cpool.tile([128, 512], f32)
            nc.vector.tensor_copy(out_sb[:, 384:512], accd[:])
            nc.sync.dma_start(out[:, 384:512], out_sb[:, 384:512])
            nc.vector.tensor_copy(out_sb[:, 0:384], acc[:])
            nc.sync.dma_start(out[:, 0:384], out_sb[:, 0:384])

    _strip_ctrl_drain_waits(nc)
    return nc


def _get_program():
    if "nc" not in _prog_cache:
        _prog_cache["nc"] = _build_program()
    return _prog_cache["nc"]


def _host_preprocess(positions, colors, opacities, scales, qvec, tvec):
    # Mirror the reference's fp32 projection math.
    q = qvec.astype(np.float32)
    q = q / np.sqrt(np.sum(q * q, dtype=np.float32)).astype(np.float32)
    w, x, y, z = q[0], q[1], q[2], q[3]
    R = np.array(
        [
            [1 - 2 * (y * y + z * z), 2 * (x * y - z * w), 2 * (x * z + y * w)],
            [2 * (x * y + z * w), 1 - 2 * (x * x + z * z), 2 * (y * z - x * w)],
            [2 * (x * z - y * w), 2 * (y * z + x * w), 1 - 2 * (x * x + y * y)],
        ],
        dtype=np.float32,
    )
    p_cam = positions.astype(np.float32) @ R.T + tvec.astype(np.float32)[None, :]
    px = p_cam[:, 0] / p_cam[:, 2] * np.float32(FX) + np.float32(CX)
    py = p_cam[:, 1] / p_cam[:, 2] * np.float32(FY) + np.float32(CY)

    var = scales[:, 0].astype(np.float32) ** 2
    iv = np.float32(-0.5) / var
    op = opacities[:, 0].astype(np.float32)

    # gp[c] = [px, py, iv, log(op), ...]; gph[c] = fp16 colors [r, g, b]
    gp = np.zeros((N, 4), dtype=np.float32)
    gp[:, 0] = px
    gp[:, 1] = py
    gp[:, 2] = iv
    # exp bias applies to both the gx and gy halves of the per-chunk
    # activation, so each factor carries sqrt(op): gx*gy = op*...
    gp[:, 3] = np.float32(0.5) * np.log(np.maximum(op, np.float32(1e-38)))
    gph = colors.astype(np.float16)
    # core/chunk/partition layout: [NCORES, NCHUNK, 128, f] -> [cores][128, NCHUNK*f]
    gp = gp.reshape(NCORES, NCHUNK, 128, 4).transpose(0, 2, 1, 3)
    gph = gph.reshape(NCORES, NCHUNK, 128, 3).transpose(0, 2, 1, 3)
    return (
        np.ascontiguousarray(gp.reshape(NCORES, 128, NCHUNK * 4)),
        np.ascontiguousarray(gph.reshape(NCORES, 128, NCHUNK * 3)),
    )


def _host_postprocess(partials):
    # partials: [NCORES, 128(y), 512] -> full image
    tot = partials.sum(axis=0, dtype=np.float32)  # [y, 512]
    num = tot[:, :384].reshape(128, 3, 128).transpose(0, 2, 1)  # [y, x, 3]
    den = tot[:, 384:512] + np.float32(REF_CHUNK_EPS)  # [y, x]
    img = num / np.maximum(den, np.float32(EPS))[:, :, None]
    img_flat = img.reshape(H * W, 3)
    step_px = 64 * 64
    n_tiles = (H * W) // step_px
    tiles = img_flat.reshape(n_tiles, step_px, 3).transpose(0, 2, 1)
    return np.ascontiguousarray(tiles.reshape(n_tiles, 3, 64, 64), dtype=np.float32)


def kernel(positions, colors, opacities, scales, qvec, tvec):
    from concourse.bass_utils import run_bass_kernel_spmd

    positions = np.asarray(positions)
    colors = np.asarray(colors)
    opacities = np.asarray(opacities)
    scales = np.asarray(scales)
    qvec = np.asarray(qvec)
    tvec = np.asarray(tvec)

    nc = _get_program()
    gp_per_core, gph_per_core = _host_preprocess(
        positions, colors, opacities, scales, qvec, tvec
    )
    in_maps = [
        {"gp": gp_per_core[i], "gph": gph_per_core[i]} for i in range(NCORES)
    ]
    res = run_bass_kernel_spmd(nc, in_maps, list(range(NCORES)))
    partials = np.stack(
        [res.results[i]["out"].astype(np.float32) for i in range(NCORES)]
    )
    return _host_postprocess(partials)


# revision 26
# speedup vs baseline: 1.0721x; 1.0502x over previous
"""Trainium2 Bass kernel for the DifferentiableRenderer problem.

Math: the reference splats N=8192 isotropic 2D gaussians onto a 128x128
pixel grid:  w[c,p] = op[c] * exp(-0.5*dist2(c,p)/var[c]),
             img = (w^T @ colors) / (sum_c w + eps).

Key algebraic restructuring: the pixel grid is separable, so
  exp(-0.5*((px-x)^2+(py-y)^2)/var) = gx[c,x] * gy[c,y]
with gx/gy 1D gaussian factors over the 128 grid coordinates.  The
accumulation then becomes, per channel d (3 colors + 1 denominator):
  num_d[y,x] = sum_c gy[c,y] * (gx*op*cl_d)[c,x]
i.e. a matmul contracting over gaussians, with only N*256 exps instead
of N*16384.

Sharding: gaussians are sharded 8 ways (1024/core); every core computes
partial (num, den) for the full image; host sums the 8 partials
(the "shard gaussians and all-reduce accumulators" strategy, with the
all-reduce realized as the host-side unshard of the partial outputs).

Device layout per core (1024 gaussians = 8 chunks of 128 partitions):
  gp  [128, 32]  fp32 params, chunk k at cols 4k..4k+3:
                 [px, py, -0.5/var, 0.5*log(op)]
  gph [128, 24]  fp16 colors, chunk k at cols 3k..3k+2: [r, g, b]
  out [128, 512] fp32 partial accumulators out[y, 128d+x], d=3 is den.
  (the pixel-grid constant xg[p, j] = j is generated on device via iota)

Pipeline per core:
  diff[c, k, {x,y}, 128] = (grid - center)^2   (DVE TTs, broadcast APs,
      2-chunk quarters so chunk 0's exp starts early)
  per chunk: gxy[c, 2, 256] = exp(iv*diff_k + log(op)/2)  fp16
      (1 ACT op per chunk; the bias hits both the gx and gy halves, so
      each carries sqrt(op))
  per chunk pair: B[c, 2, 3, 128] = gx * rgb   (1 DVE TT, broadcast APs)
  per chunk: accd[y, 128]  += gy^T @ gx        (PE fp16, den PSUM bank)
             acc[y, 384]   += gy^T @ B_k       (PE fp16, num PSUM bank)
  copy PSUM->SBUF, den half DMA'd out early, num half after last matmul.
"""

import numpy as np

H = W = 128
FX = FY = 150.0
CX = CY = 64.0
N = 8192
NCORES = 8
NPC = N // NCORES          # gaussians per core
NCHUNK = NPC // 128        # 128-gaussian chunks per core
REF_CHUNK_EPS = (N // 2048) * 1e-8  # reference adds EPS to den once per chunk step
EPS = 1e-8

_prog_cache = {}


def _make_split_wait_tile_context(tile_mod):
    """TileContext whose commit stage splits multi-wait instructions.

    The walrus build used on the axon path allows at most ONE sync wait
    per instruction (any engine/format).  Tile's add_semaphores pass
    freely emits 2-3 waits per instruction, so at commit time we peel
    all but the last wait onto standalone EventSemaphore instructions
    appended just before the real one on the same engine queue —
    semantically identical (waits on one queue execute in order)."""
    import bass_rust
    from concourse import mybir

    class SplitWaitTileContext(tile_mod.TileContext):
        def _add_instruction(self, inst):
            si = inst.sync_info
            if si is not None and si.on_wait and len(si.on_wait) > 1:
                waits = list(si.on_wait)
                for j, w in enumerate(waits[:-1]):
                    carrier = mybir.InstEventSemaphore(
                        name=f"{inst.name}-sw{j}",
                        ins=[],
                        outs=[],
                        engine=inst.engine,
                        sync_info=bass_rust.SyncInfo(on_wait=[w], on_update=[]),
                    )
                    super()._add_instruction(carrier)
                si.on_wait = [waits[-1]]
            super()._add_instruction(inst)

    return SplitWaitTileContext


def _strip_ctrl_drain_waits(nc):
    # The tail drain aggregates one wait per open semaphore, but CTRL
    # instructions accept only one; the waits are redundant at kernel
    # end (the all-engine barrier that follows drains every engine and
    # its DMA rings before semaphores are cleared).  Keep one.
    for blk in nc.m.functions[0].blocks:
        for inst in blk.instructions:
            si = inst.sync_info
            if (
                si is not None
                and si.on_wait
                and len(si.on_wait) > 1
                and type(inst).__name__ == "InstDrain"
            ):
                waits = list(si.on_wait)
                keep = [w for w in waits if not w.ant_name.startswith("DMA")][:1]
                if not keep:
                    keep = waits[:1]
                si.on_wait = keep


def _build_program():
    import concourse.bass as bass
    import concourse.tile as tile
    from concourse import mybir

    f32 = mybir.dt.float32
    f16 = mybir.dt.float16
    Exp = mybir.ActivationFunctionType.Exp
    Square = mybir.ActivationFunctionType.Square
    add_op = mybir.AluOpType.add
    mul_op = mybir.AluOpType.mult

    nc = bass.Bass(debug=False)
    gp = nc.dram_tensor("gp", [128, 4 * NCHUNK], f32, kind="ExternalInput")
    gph = nc.dram_tensor("gph", [128, 3 * NCHUNK], f16, kind="ExternalInput")
    out = nc.dram_tensor("out", [128, 512], f32, kind="ExternalOutput")

    TC = _make_split_wait_tile_context(tile)
    with TC(nc) as tc:
        with (
            tc.tile_pool(name="const", bufs=1) as cpool,
            tc.tile_pool(name="work", bufs=4) as wpool,
            tc.tile_pool(name="psum", bufs=1, space="PSUM") as ppool,
        ):
            # Dummy exp on a const input: pulls the ACT exp-table load off
            # the critical path (it otherwise serializes before chunk 0).
            warm = cpool.tile([128, 1], f32)
            nc.scalar.activation(
                warm[:], nc.const_aps.scalar_like(0.0, warm[:]), Exp
            )

            gp_t = cpool.tile([128, 4 * NCHUNK], f32)
            nc.sync.dma_start(gp_t[:], gp[:])
            # grid constant generated on device: xg[p, j] = j  (before the
            # gph DMA on the gpsimd queue: xg gates the first sub, gph
            # is not needed until the first B-form)
            xg_t = cpool.tile([128, 128], f32)
            nc.gpsimd.iota(
                xg_t[:],
                pattern=[[1, 128]],
                channel_multiplier=0,
                allow_small_or_imprecise_dtypes=True,
            )
            gph_t = cpool.tile([128, 3 * NCHUNK], f16)
            nc.gpsimd.dma_start(gph_t[:], gph[:])

            gp_v = gp_t[:].rearrange("p (k f) -> p k f", f=4)  # [128, 8, 4]

            # diff[c, k, {x,y}, 128] = (grid - center)^2.  Centers are
            # stored NEGATED so (a) the DVE path is a tensor_tensor add
            # (broadcast APs) and (b) ScalarE can compute whole squared
            # diffs in ONE op each via Square(grid + bias) — the Exp and
            # Square tables are co-resident, so mixing them is free.
            # Split: x-axis of chunks 0-3 on ScalarE (otherwise idle
            # before its exp chain), everything else on DVE.
            diff = cpool.tile([128, NCHUNK, 2, 128], f32)
            QC = 2
            ACT_X = 4  # chunks whose x-axis square runs on ScalarE
            for h in range(ACT_X // QC):
                ks = slice(h * QC, (h + 1) * QC)
                nc.vector.tensor_tensor(
                    diff[:, ks, 1:2],
                    xg_t[:, None, None, :].broadcast_to([128, QC, 1, 128]),
                    gp_v[:, ks, 1:2, None].broadcast_to([128, QC, 1, 128]),
                    add_op,
                )
                nc.vector.tensor_tensor(
                    diff[:, ks, 1:2], diff[:, ks, 1:2], diff[:, ks, 1:2], mul_op
                )
            for k in range(ACT_X):
                nc.scalar.activation(
                    diff[:, k, 0],
                    xg_t[:],
                    Square,
                    bias=gp_t[:, 4 * k : 4 * k + 1],
                )
            for h in range(ACT_X // QC, NCHUNK // QC):
                ks = slice(h * QC, (h + 1) * QC)
                nc.vector.tensor_tensor(
                    diff[:, ks],
                    xg_t[:, None, None, :].broadcast_to([128, QC, 2, 128]),
                    gp_v[:, ks, 0:2, None].broadcast_to([128, QC, 2, 128]),
                    add_op,
                )
                nc.vector.tensor_tensor(diff[:, ks], diff[:, ks], diff[:, ks], mul_op)

            gph_v = gph_t[:].rearrange("p (k f) -> p k f", f=3)  # [128, 8, 3]
            acc = ppool.tile([128, 384], f32)
            accd = ppool.tile([128, 128], f32)
            for k2 in range(NCHUNK // 2):
                # fp16 gaussian factors; opacity folded into gy via the
                # log-opacity activation bias, so B only needs the 3 color
                # channels and the den channel is gx itself (2nd matmul).
                gxy = wpool.tile([128, 2, 256], f16, tag="gxy")
                for j in range(2):
                    k = 2 * k2 + j
                    nc.scalar.activation(
                        gxy[:, j],
                        diff[:, k].rearrange("p a b -> p (a b)"),
                        Exp,
                        bias=gp_t[:, 4 * k + 3 : 4 * k + 4],
                        scale=gp_t[:, 4 * k + 2 : 4 * k + 3],
                    )
                B = wpool.tile([128, 2, 3, 128], f16, tag="B")
                nc.vector.tensor_tensor(
                    B[:],
                    gxy[:, :, None, 0:128].broadcast_to([128, 2, 3, 128]),
                    gph_v[:, 2 * k2 : 2 * k2 + 2, :, None].broadcast_to(
                        [128, 2, 3, 128]
                    ),
                    mul_op,
                )
                for j in range(2):
                    k = 2 * k2 + j
                    gy = gxy[:, j, 128:256]
                    nc.tensor.matmul(
                        accd[:],
                        gy,
                        gxy[:, j, 0:128],
                        start=(k == 0),
                        stop=(k == NCHUNK - 1),
                    )
                    nc.tensor.matmul(
                        acc[:],
                        gy,
                        B[:, j].rearrange("p a b -> p (a b)"),
                        start=(k == 0),
                        stop=(k == NCHUNK - 1),
                    )

            # fp16 partials out; den finishes ~2us before num (its last
            # accumulating matmul is earlier in the PE queue), so copy +
            # DMA it while the num matmuls still run.
            out_sb = cpool.tile([128, 512], f32)
            nc.vector.tensor_copy(out_sb[:, 384:512], accd[:])
            nc.sync.dma_start(out[:, 384:512], out_sb[:, 384:512])
            nc.vector.tensor_copy(out_sb[:, 0:384], acc[:])
            nc.sync.dma_start(out[:, 0:384], out_sb[:, 0:384])

    _strip_ctrl_drain_waits(nc)
    return nc


def _get_program():
    if "nc" not in _prog_cache:
        _prog_cache["nc"] = _build_program()
    return _prog_cache["nc"]


def _host_preprocess(positions, colors, opacities, scales, qvec, tvec):
    # Mirror the reference's fp32 projection math.
    q = qvec.astype(np.float32)
    q = q / np.sqrt(np.sum(q * q, dtype=np.float32)).astype(np.float32)
    w, x, y, z = q[0], q[1], q[2], q[3]
    R = np.array(
        [
            [1 - 2 * (y * y + z * z), 2 * (x * y - z * w), 2 * (x * z + y * w)],
            [2 * (x * y + z * w), 1 - 2 * (x * x + z * z), 2 * (y * z - x * w)],
            [2 * (x * z - y * w), 2 * (y * z + x * w), 1 - 2 * (x * x + y * y)],
        ],
        dtype=np.float32,
    )
    p_cam = positions.astype(np.float32) @ R.T + tvec.astype(np.float32)[None, :]
    px = p_cam[:, 0] / p_cam[:, 2] * np.float32(FX) + np.float32(CX)
    py = p_cam[:, 1] / p_cam[:, 2] * np.float32(FY) + np.float32(CY)

    var = scales[:, 0].astype(np.float32) ** 2
    iv = np.float32(-0.5) / var
    op = opacities[:, 0].astype(np.float32)

    # gp[c] = [px, py, iv, log(op), ...]; gph[c] = fp16 colors [r, g, b]
    gp = np.zeros((N, 4), dtype=np.float32)
    gp[:, 0] = -px
    gp[:, 1] = -py
    gp[:, 2] = iv
    # exp bias applies to both the gx and gy halves of the per-chunk
    # activation, so each factor carries sqrt(op): gx*gy = op*...
    gp[:, 3] = np.float32(0.5) * np.log(np.maximum(op, np.float32(1e-38)))
    gph = colors.astype(np.float16)
    # core/chunk/partition layout: [NCORES, NCHUNK, 128, f] -> [cores][128, NCHUNK*f]
    gp = gp.reshape(NCORES, NCHUNK, 128, 4).transpose(0, 2, 1, 3)
    gph = gph.reshape(NCORES, NCHUNK, 128, 3).transpose(0, 2, 1, 3)
    return (
        np.ascontiguousarray(gp.reshape(NCORES, 128, NCHUNK * 4)),
        np.ascontiguousarray(gph.reshape(NCORES, 128, NCHUNK * 3)),
    )


def _host_postprocess(partials):
    # partials: [NCORES, 128(y), 512] -> full image
    tot = partials.sum(axis=0, dtype=np.float32)  # [y, 512]
    num = tot[:, :384].reshape(128, 3, 128).transpose(0, 2, 1)  # [y, x, 3]
    den = tot[:, 384:512] + np.float32(REF_CHUNK_EPS)  # [y, x]
    img = num / np.maximum(den, np.float32(EPS))[:, :, None]
    img_flat = img.reshape(H * W, 3)
    step_px = 64 * 64
    n_tiles = (H * W) // step_px
    tiles = img_flat.reshape(n_tiles, step_px, 3).transpose(0, 2, 1)
    return np.ascontiguousarray(tiles.reshape(n_tiles, 3, 64, 64), dtype=np.float32)


def kernel(positions, colors, opacities, scales, qvec, tvec):
    from concourse.bass_utils import run_bass_kernel_spmd

    positions = np.asarray(positions)
    colors = np.asarray(colors)
    opacities = np.asarray(opacities)
    scales = np.asarray(scales)
    qvec = np.asarray(qvec)
    tvec = np.asarray(tvec)

    nc = _get_program()
    gp_per_core, gph_per_core = _host_preprocess(
        positions, colors, opacities, scales, qvec, tvec
    )
    in_maps = [
        {"gp": gp_per_core[i], "gph": gph_per_core[i]} for i in range(NCORES)
    ]
    res = run_bass_kernel_spmd(nc, in_maps, list(range(NCORES)))
    partials = np.stack(
        [res.results[i]["out"].astype(np.float32) for i in range(NCORES)]
    )
    return _host_postprocess(partials)


# revision 27
# speedup vs baseline: 1.1000x; 1.0261x over previous
"""Trainium2 Bass kernel for the DifferentiableRenderer problem.

Math: the reference splats N=8192 isotropic 2D gaussians onto a 128x128
pixel grid:  w[c,p] = op[c] * exp(-0.5*dist2(c,p)/var[c]),
             img = (w^T @ colors) / (sum_c w + eps).

Key algebraic restructuring: the pixel grid is separable, so
  exp(-0.5*((px-x)^2+(py-y)^2)/var) = gx[c,x] * gy[c,y]
with gx/gy 1D gaussian factors over the 128 grid coordinates.  The
accumulation then becomes, per channel d (3 colors + 1 denominator):
  num_d[y,x] = sum_c gy[c,y] * (gx*op*cl_d)[c,x]
i.e. a matmul contracting over gaussians, with only N*256 exps instead
of N*16384.

Sharding: gaussians are sharded 8 ways (1024/core); every core computes
partial (num, den) for the full image; host sums the 8 partials
(the "shard gaussians and all-reduce accumulators" strategy, with the
all-reduce realized as the host-side unshard of the partial outputs).

Device layout per core (1024 gaussians = 8 chunks of 128 partitions):
  gp  [128, 32]  fp32 params, chunk k at cols 4k..4k+3:
                 [px, py, -0.5/var, 0.5*log(op)]
  gph [128, 24]  fp16 colors, chunk k at cols 3k..3k+2: [r, g, b]
  out [128, 512] fp32 partial accumulators out[y, 128d+x], d=3 is den.
  (the pixel-grid constant xg[p, j] = j is generated on device via iota)

Pipeline per core:
  diff[c, k, {x,y}, 128] = (grid - center)^2   (DVE TTs, broadcast APs,
      2-chunk quarters so chunk 0's exp starts early)
  per chunk: gxy[c, 2, 256] = exp(iv*diff_k + log(op)/2)  fp16
      (1 ACT op per chunk; the bias hits both the gx and gy halves, so
      each carries sqrt(op))
  per chunk pair: B[c, 2, 3, 128] = gx * rgb   (1 DVE TT, broadcast APs)
  per chunk: accd[y, 128]  += gy^T @ gx        (PE fp16, den PSUM bank)
             acc[y, 384]   += gy^T @ B_k       (PE fp16, num PSUM bank)
  copy PSUM->SBUF, den half DMA'd out early, num half after last matmul.
"""

import numpy as np

H = W = 128
FX = FY = 150.0
CX = CY = 64.0
N = 8192
NCORES = 8
NPC = N // NCORES          # gaussians per core
NCHUNK = NPC // 128        # 128-gaussian chunks per core
REF_CHUNK_EPS = (N // 2048) * 1e-8  # reference adds EPS to den once per chunk step
EPS = 1e-8

_prog_cache = {}


def _make_split_wait_tile_context(tile_mod):
    """TileContext whose commit stage splits multi-wait instructions.

    The walrus build used on the axon path allows at most ONE sync wait
    per instruction (any engine/format).  Tile's add_semaphores pass
    freely emits 2-3 waits per instruction, so at commit time we peel
    all but the last wait onto standalone EventSemaphore instructions
    appended just before the real one on the same engine queue —
    semantically identical (waits on one queue execute in order)."""
    import bass_rust
    from concourse import mybir

    class SplitWaitTileContext(tile_mod.TileContext):
        def _add_instruction(self, inst):
            si = inst.sync_info
            if si is not None and si.on_wait and len(si.on_wait) > 1:
                waits = list(si.on_wait)
                for j, w in enumerate(waits[:-1]):
                    carrier = mybir.InstEventSemaphore(
                        name=f"{inst.name}-sw{j}",
                        ins=[],
                        outs=[],
                        engine=inst.engine,
                        sync_info=bass_rust.SyncInfo(on_wait=[w], on_update=[]),
                    )
                    super()._add_instruction(carrier)
                si.on_wait = [waits[-1]]
            super()._add_instruction(inst)

    return SplitWaitTileContext


def _strip_ctrl_drain_waits(nc):
    # The tail drain aggregates one wait per open semaphore, but CTRL
    # instructions accept only one; the waits are redundant at kernel
    # end (the all-engine barrier that follows drains every engine and
    # its DMA rings before semaphores are cleared).  Keep one.
    for blk in nc.m.functions[0].blocks:
        for inst in blk.instructions:
            si = inst.sync_info
            if (
                si is not None
                and si.on_wait
                and len(si.on_wait) > 1
                and type(inst).__name__ == "InstDrain"
            ):
                waits = list(si.on_wait)
                keep = [w for w in waits if not w.ant_name.startswith("DMA")][:1]
                if not keep:
                    keep = waits[:1]
                si.on_wait = keep


def _build_program():
    import concourse.bass as bass
    import concourse.tile as tile
    from concourse import mybir

    f32 = mybir.dt.float32
    f16 = mybir.dt.float16
    Exp = mybir.ActivationFunctionType.Exp
    Square = mybir.ActivationFunctionType.Square
    add_op = mybir.AluOpType.add
    mul_op = mybir.AluOpType.mult

    nc = bass.Bass(debug=False)
    gp = nc.dram_tensor("gp", [128, 4 * NCHUNK], f32, kind="ExternalInput")
    gph = nc.dram_tensor("gph", [128, 3 * NCHUNK], f16, kind="ExternalInput")
    out = nc.dram_tensor("out", [128, 512], f32, kind="ExternalOutput")

    TC = _make_split_wait_tile_context(tile)
    with TC(nc) as tc:
        with (
            tc.tile_pool(name="const", bufs=1) as cpool,
            tc.tile_pool(name="work", bufs=4) as wpool,
            tc.tile_pool(name="psum", bufs=1, space="PSUM") as ppool,
        ):
            # Dummy exp on a const input: pulls the ACT exp-table load off
            # the critical path (it otherwise serializes before chunk 0).
            warm = cpool.tile([128, 1], f32)
            nc.scalar.activation(
                warm[:], nc.const_aps.scalar_like(0.0, warm[:]), Exp
            )

            gp_t = cpool.tile([128, 4 * NCHUNK], f32)
            nc.sync.dma_start(gp_t[:], gp[:])
            # grid constant generated on device: xg[p, j] = j  (before the
            # gph DMA on the gpsimd queue: xg gates the first sub, gph
            # is not needed until the first B-form)
            xg_t = cpool.tile([128, 128], f32)
            nc.gpsimd.iota(
                xg_t[:],
                pattern=[[1, 128]],
                channel_multiplier=0,
                allow_small_or_imprecise_dtypes=True,
            )
            gph_t = cpool.tile([128, 3 * NCHUNK], f16)
            nc.gpsimd.dma_start(gph_t[:], gph[:])

            gp_v = gp_t[:].rearrange("p (k f) -> p k f", f=4)  # [128, 8, 4]

            # diff[c, k, {x,y}, 128] = (grid - center)^2.  Centers are
            # stored NEGATED so (a) the DVE path is a tensor_tensor add
            # (broadcast APs) and (b) ScalarE can compute whole squared
            # diffs in ONE op each via Square(grid + bias) — the Exp and
            # Square tables are co-resident, so mixing them is free.
            # Split: x-axis of chunks 0-3 on ScalarE (otherwise idle
            # before its exp chain), everything else on DVE.
            diff = cpool.tile([128, NCHUNK, 2, 128], f32)
            QC = 2
            ACT_X = 6  # chunks whose x-axis square runs on ScalarE
            for h in range(ACT_X // QC):
                ks = slice(h * QC, (h + 1) * QC)
                nc.vector.tensor_tensor(
                    diff[:, ks, 1:2],
                    xg_t[:, None, None, :].broadcast_to([128, QC, 1, 128]),
                    gp_v[:, ks, 1:2, None].broadcast_to([128, QC, 1, 128]),
                    add_op,
                )
                nc.vector.tensor_tensor(
                    diff[:, ks, 1:2], diff[:, ks, 1:2], diff[:, ks, 1:2], mul_op
                )
            for k in range(ACT_X):
                nc.scalar.activation(
                    diff[:, k, 0],
                    xg_t[:],
                    Square,
                    bias=gp_t[:, 4 * k : 4 * k + 1],
                )
            for h in range(ACT_X // QC, NCHUNK // QC):
                ks = slice(h * QC, (h + 1) * QC)
                nc.vector.tensor_tensor(
                    diff[:, ks],
                    xg_t[:, None, None, :].broadcast_to([128, QC, 2, 128]),
                    gp_v[:, ks, 0:2, None].broadcast_to([128, QC, 2, 128]),
                    add_op,
                )
                nc.vector.tensor_tensor(diff[:, ks], diff[:, ks], diff[:, ks], mul_op)

            gph_v = gph_t[:].rearrange("p (k f) -> p k f", f=3)  # [128, 8, 3]
            acc = ppool.tile([128, 384], f32)
            accd = ppool.tile([128, 128], f32)
            for k2 in range(NCHUNK // 2):
                # fp16 gaussian factors; opacity folded into gy via the
                # log-opacity activation bias, so B only needs the 3 color
                # channels and the den channel is gx itself (2nd matmul).
                gxy = wpool.tile([128, 2, 256], f16, tag="gxy")
                for j in range(2):
                    k = 2 * k2 + j
                    nc.scalar.activation(
                        gxy[:, j],
                        diff[:, k].rearrange("p a b -> p (a b)"),
                        Exp,
                        bias=gp_t[:, 4 * k + 3 : 4 * k + 4],
                        scale=gp_t[:, 4 * k + 2 : 4 * k + 3],
                    )
                B = wpool.tile([128, 2, 3, 128], f16, tag="B")
                nc.vector.tensor_tensor(
                    B[:],
                    gxy[:, :, None, 0:128].broadcast_to([128, 2, 3, 128]),
                    gph_v[:, 2 * k2 : 2 * k2 + 2, :, None].broadcast_to(
                        [128, 2, 3, 128]
                    ),
                    mul_op,
                )
                for j in range(2):
                    k = 2 * k2 + j
                    gy = gxy[:, j, 128:256]
                    nc.tensor.matmul(
                        accd[:],
                        gy,
                        gxy[:, j, 0:128],
                        start=(k == 0),
                        stop=(k == NCHUNK - 1),
                    )
                    nc.tensor.matmul(
                        acc[:],
                        gy,
                        B[:, j].rearrange("p a b -> p (a b)"),
                        start=(k == 0),
                        stop=(k == NCHUNK - 1),
                    )

            # fp16 partials out; den finishes ~2us before num (its last
            # accumulating matmul is earlier in the PE queue), so copy +
            # DMA it while the num matmuls still run.
            out_sb = cpool.tile([128, 512], f32)
            nc.vector.tensor_copy(out_sb[:, 384:512], accd[:])
            nc.sync.dma_start(out[:, 384:512], out_sb[:, 384:512])
            nc.vector.tensor_copy(out_sb[:, 0:384], acc[:])
            nc.sync.dma_start(out[:, 0:384], out_sb[:, 0:384])

    _strip_ctrl_drain_waits(nc)
    return nc


def _get_program():
    if "nc" not in _prog_cache:
        _prog_cache["nc"] = _build_program()
    return _prog_cache["nc"]


def _host_preprocess(positions, colors, opacities, scales, qvec, tvec):
    # Mirror the reference's fp32 projection math.
    q = qvec.astype(np.float32)
    q = q / np.sqrt(np.sum(q * q, dtype=np.float32)).astype(np.float32)
    w, x, y, z = q[0], q[1], q[2], q[3]
    R = np.array(
        [
            [1 - 2 * (y * y + z * z), 2 * (x * y - z * w), 2 * (x * z + y * w)],
            [2 * (x * y + z * w), 1 - 2 * (x * x + z * z), 2 * (y * z - x * w)],
            [2 * (x * z - y * w), 2 * (y * z + x * w), 1 - 2 * (x * x + y * y)],
        ],
        dtype=np.float32,
    )
    p_cam = positions.astype(np.float32) @ R.T + tvec.astype(np.float32)[None, :]
    px = p_cam[:, 0] / p_cam[:, 2] * np.float32(FX) + np.float32(CX)
    py = p_cam[:, 1] / p_cam[:, 2] * np.float32(FY) + np.float32(CY)

    var = scales[:, 0].astype(np.float32) ** 2
    iv = np.float32(-0.5) / var
    op = opacities[:, 0].astype(np.float32)

    # gp[c] = [px, py, iv, log(op), ...]; gph[c] = fp16 colors [r, g, b]
    gp = np.zeros((N, 4), dtype=np.float32)
    gp[:, 0] = -px
    gp[:, 1] = -py
    gp[:, 2] = iv
    # exp bias applies to both the gx and gy halves of the per-chunk
    # activation, so each factor carries sqrt(op): gx*gy = op*...
    gp[:, 3] = np.float32(0.5) * np.log(np.maximum(op, np.float32(1e-38)))
    gph = colors.astype(np.float16)
    # core/chunk/partition layout: [NCORES, NCHUNK, 128, f] -> [cores][128, NCHUNK*f]
    gp = gp.reshape(NCORES, NCHUNK, 128, 4).transpose(0, 2, 1, 3)
    gph = gph.reshape(NCORES, NCHUNK, 128, 3).transpose(0, 2, 1, 3)
    return (
        np.ascontiguousarray(gp.reshape(NCORES, 128, NCHUNK * 4)),
        np.ascontiguousarray(gph.reshape(NCORES, 128, NCHUNK * 3)),
    )


def _host_postprocess(partials):
    # partials: [NCORES, 128(y), 512] -> full image
    tot = partials.sum(axis=0, dtype=np.float32)  # [y, 512]
    num = tot[:, :384].reshape(128, 3, 128).transpose(0, 2, 1)  # [y, x, 3]
    den = tot[:, 384:512] + np.float32(REF_CHUNK_EPS)  # [y, x]
    img = num / np.maximum(den, np.float32(EPS))[:, :, None]
    img_flat = img.reshape(H * W, 3)
    step_px = 64 * 64
    n_tiles = (H * W) // step_px
    tiles = img_flat.reshape(n_tiles, step_px, 3).transpose(0, 2, 1)
    return np.ascontiguousarray(tiles.reshape(n_tiles, 3, 64, 64), dtype=np.float32)


def kernel(positions, colors, opacities, scales, qvec, tvec):
    from concourse.bass_utils import run_bass_kernel_spmd

    positions = np.asarray(positions)
    colors = np.asarray(colors)
    opacities = np.asarray(opacities)
    scales = np.asarray(scales)
    qvec = np.asarray(qvec)
    tvec = np.asarray(tvec)

    nc = _get_program()
    gp_per_core, gph_per_core = _host_preprocess(
        positions, colors, opacities, scales, qvec, tvec
    )
    in_maps = [
        {"gp": gp_per_core[i], "gph": gph_per_core[i]} for i in range(NCORES)
    ]
    res = run_bass_kernel_spmd(nc, in_maps, list(range(NCORES)))
    partials = np.stack(
        [res.results[i]["out"].astype(np.float32) for i in range(NCORES)]
    )
    return _host_postprocess(partials)
